# revision 30
# baseline (speedup 1.0000x reference)
"""Trainium2 Bass kernel for nn_CDA_attention (density-modulated attention).

Contract: kernel(**full_inputs) -> full output [8, 256, 64, 64] float32.
Data-parallel over batch: core b computes batch b.

Per-core computation (batch b, C=256, N=4096):
  - scores fold Wk into the query projection: score = x8_k^T z_q with
    z = (Wq^T Wk) x + Wk^T bq, so the QK^T stationary is x8 itself (no
    k-projection, no k8 SBUF tile, no k eviction traffic).  The per-key
    temperature skv is applied INSIDE the exp as a per-partition ACT
    activation scale / per-partition DVE Schraudolph multiplier, so the
    density chain gates only the exps, never the score GEMMs.
  - all GEMMs run fp8e4 DoubleRow (K=256/pass): scores, attn@V
    (vproj = (Wout@Wv) x with a ones column -> row sums), z projection.
  - density chain: gray = mean_c x via col-tiled M=1 matmuls; Laplacian
    + abs on DVE in [64,64] image layout; conv1/conv2 as single-pass
    im2col matmuls (K=9 / K=72) over row/col-shifted flat copies
    (af9 / h72), col-tiled 4-wide; sigmoid/recip/exp-scales in a
    [128, 32] = [key-in-chunk, chunk] layout.
  - exp per key-chunk split across two engines: ACT true exp
    (scale=skv/16) and DVE one-op Schraudolph (fp8 bits =
    score*(8*log2e/16*skv) + const, saturating f32->uint8 convert
    bitcast to fp8e4).
  - attn@vproj rowsum-normalized, bf16-transposed back to [c, n] on the
    PE, + fused bias + residual.
"""

import os
import sys

sys.path.insert(0, "/opt/trn_rl_repo")

from contextlib import ExitStack

import ml_dtypes
import numpy as np

import concourse.bass as bass
import concourse.mybir as mybir
import concourse.tile as tile
from concourse import bacc, bass_utils
from concourse.masks import make_identity

B, C, HH, WW = 8, 256, 64, 64
N = HH * WW          # 4096
P = 128
CC = C // P          # 2 channel chunks
NQT = 512            # query tile (columns per score matmul)
NQ_TILES = N // NQT  # 8
NKC = N // P         # 32 key chunks
NPAIR = NKC // 2     # 16 key-chunk pairs
NSUB = NQT // P      # 4 query sub-tiles per query tile

f32 = mybir.dt.float32
bf16 = mybir.dt.bfloat16
f8 = mybir.dt.float8e4
u8 = mybir.dt.uint8
DR = mybir.MatmulPerfMode.DoubleRow
AF = mybir.ActivationFunctionType
ALU = mybir.AluOpType

# exps on DVE (Schraudolph) vs ACT: chunk-level map for qtile 0 (gated on
# the short [128,32] skv path), pair-level map for qtiles 1+ (skv in x8s)
DVE_CHUNKS = frozenset(j for j in range(NKC) if j % 2 == 1)
DVE_PAIRS = frozenset(j for j in range(NPAIR) if j % 2 == 1)
AV_LAG = int(os.environ.get("KERNEL_AV_LAG", "4"))
N_WARM = int(os.environ.get("KERNEL_WARM", "20"))
# fp8e4 Schraudolph constants: bits = score*skv*SCH_A + SCH_B (f32->uint8,
# truncating convert => +0.5; -0.475 centers the mantissa-interp bias)
SCH_A = 8.0 * 1.4426950408889634 / 16.0
SCH_B = 56.0 - 2.0 * 8.0 * 1.4426950408889634 - 0.475 + 0.5


def build_kernel_body(tc, ctx, d, has_r):
    nc = tc.nc
    x_d, x8_d = d["x"], d["x8"]
    out_d, scr1, scr2 = d["out"], d["scr1"], d["scr2"]
    scr2b = d["scr2b"]
    scr1_2d = scr1.rearrange("(a b) -> a b", a=1)
    scr2_2d = scr2.rearrange("(a b) -> a b", a=1)

    const = ctx.enter_context(tc.tile_pool(name="const", bufs=1))
    big = ctx.enter_context(tc.tile_pool(name="big", bufs=1))
    ps_pool = ctx.enter_context(tc.tile_pool(name="ps", bufs=2, space="PSUM"))
    po_pool = ctx.enter_context(tc.tile_pool(name="po", bufs=2, space="PSUM"))
    fin_pool = ctx.enter_context(tc.tile_pool(name="fin", bufs=2))
    osb_pool = ctx.enter_context(tc.tile_pool(name="osb", bufs=2))
    rcp_pool = ctx.enter_context(tc.tile_pool(name="rcp", bufs=2))
    qt_pool = ctx.enter_context(tc.tile_pool(name="qt", bufs=8))

    # ---- persistent SBUF tiles ----
    x_sb = big.tile([P, CC, N], f32, name="x_sb")
    x8_sb = big.tile([P, CC, N], f8, name="x8_sb")

    def xsl(tile_, start, size):
        return tile_[:, :, start:start + size]

    vproj_sb = big.tile([P, NKC, C + 1], f8)
    exp_a = big.tile([P, NKC, NQT], f8)
    exp_b = big.tile([P, NKC, NQT], f8)
    wf8_sb = const.tile([P, CC, 2 * C], f8)   # [wG | wvo] fused
    wg8_sb = wf8_sb[:, :, 0:C]
    wvo8_sb = wf8_sb[:, :, C:2 * C]
    qb4_sb = const.tile([P, 4], f32)          # [zb(2) | bfin(2)] fused
    zb_sb = qb4_sb[:, 0:2]
    bfin_sb = qb4_sb[:, 2:4]
    negb_sb = const.tile([P, 1], f32)      # -2.0 exp-bias column
    ones8_sb = const.tile([P, CC, 1], f8)  # 1/C column for the channel mean
    w3x_sb = [const.tile([3, 8], bf16, name=f"w3x{i}") for i in range(3)]
    w24x_sb = [const.tile([24, 1], bf16, name=f"w24x{i}") for i in range(3)]
    w1b128_sb = const.tile([P, 1], f32)    # conv1 bias at partition 32j+oc
    w2b128_sb = const.tile([P, 1], f32)    # conv2 bias (replicated)
    ident_bf = const.tile([P, P], bf16)
    gray_img = const.tile([64, 64], bf16)
    g_p1 = const.tile([64, 64], bf16)
    g_m1 = const.tile([64, 64], bf16)
    lap_t = const.tile([64, 64], bf16)
    abs_bf = const.tile([64, 64], bf16)
    af3 = const.tile([3, 2 + N], bf16)     # 3 row-shifted |lap| copies
    h24 = const.tile([24, 2 + N], bf16)    # dy=0 block + 2 shifted copies
    gray_flat = const.tile([33, N // 2], bf16)
    dsum_flat = const.tile([33, N // 2], bf16)
    dsum_img = const.tile([64, 64], bf16)
    sig_img = const.tile([64, 64], f32)
    skv_imgbf = const.tile([64, 64], bf16)
    skv_bc = big.tile([P, 1, N], bf16)     # skv broadcast down partitions
    x8s_sb = big.tile([P, CC, N], f8, name="x8s")  # x8 * skv (qtiles 1+)
    r_sb = const.tile([P, NQ_TILES, NQT], f32) if has_r else None

    # ---- input DMAs: x8 first (ONE descriptor -- the sync engine issues
    # descriptors serially at ~0.7us each), then the small weights ----
    x8_src = x8_d.rearrange("(c p) n -> p c n", p=P)
    nc.sync.dma_start(x8_sb[:, :, 0:N // 2], x8_src[:, :, 0:N // 2])
    nc.sync.dma_start(x8_sb[:, :, N // 2:N], x8_src[:, :, N // 2:N])
    nc.sync.dma_start(
        wf8_sb[:, :, :], d["wf8"].rearrange("(c p) w -> p c w", p=P))
    nc.sync.dma_start(qb4_sb[:, :], d["qb4"][:, :])
    for i in range(3):
        nc.sync.dma_start(w3x_sb[i][:, :], d["w9"][3 * i:3 * i + 3, :])
        nc.sync.dma_start(w24x_sb[i][:, :], d["w72"][24 * i:24 * i + 24, :])
    nc.sync.dma_start(w1b128_sb[:, :], d["w1b128"][:, :])
    nc.sync.dma_start(w2b128_sb[:, :], d["w2b128"][:, :])

    make_identity(nc, ident_bf)
    nc.gpsimd.memset(ones8_sb[:], 1.0 / C)
    nc.gpsimd.memset(negb_sb[:], -2.0)
    nc.gpsimd.memset(vproj_sb[:, :, C:C + 1], 1.0)    # ones column -> row sums
    nc.gpsimd.memset(g_p1[:], 0.0)
    nc.gpsimd.memset(g_m1[:], 0.0)
    # only the shifted-copy EDGE columns need zeroing (writes cover the rest)
    nc.vector.memset(af3[:, 0:66], 0.0)
    nc.vector.memset(af3[:, 4032:2 + N], 0.0)
    nc.vector.memset(h24[:, 0:66], 0.0)
    nc.vector.memset(h24[:, 4032:2 + N], 0.0)

    tblw = const.tile([1, 1], f32)
    nc.scalar.activation(tblw[0:1, 0:1], negb_sb[0:1, 0:1], AF.Exp,
                         bias=negb_sb[0:1, 0:1], scale=1.0)
    nc.scalar.activation(tblw[0:1, 0:1], negb_sb[0:1, 0:1], AF.Sigmoid,
                         bias=negb_sb[0:1, 0:1])
    nc.scalar.activation(tblw[0:1, 0:1], negb_sb[0:1, 0:1], AF.Relu,
                         bias=negb_sb[0:1, 0:1])

    # ---- PE warm-up: HAM unthrottles after ~3.4us of sustained activity
    # and re-throttles after ~3.4us idle; these dep-free identity matmuls
    # bridge the PE-idle stretches of the density chain.
    def pe_warm(n):
        for _ in range(n):
            dmy = ps_pool.tile([P, P], f32, tag="ps", name="warm")
            nc.tensor.matmul(dmy[:, :], ident_bf[:, :], ident_bf[:, :],
                             start=True, stop=True)

    pe_warm(N_WARM)

    # ---- gray = mean_c x: col-tiled M=1 matmuls (4 concurrent col groups)
    pgs = []
    for g in range(2):
        pg = po_pool.tile([P, NQT], f32, tag="po", name=f"pg{g}")
        for j in range(4):
            nt = 4 * g + j
            for ci in range(CC):
                nc.tensor.matmul(
                    pg[32 * j:32 * j + 1, :], ones8_sb[:, ci, :],
                    xsl(x8_sb, nt * NQT, NQT)[:, ci, :],
                    start=(ci == 0), stop=(ci == CC - 1),
                    tile_position=(0, 32 * j))
        pgs.append(pg)
    # evicts alternate ACT/DVE; per-nt 1KB scr1 writes pipeline behind them
    # (1-partition DMA runs ~1.5 GB/s -- small pipelined transfers only)
    for nt in range(NQ_TILES):
        g, j = nt // 4, nt % 4
        psrc = pgs[g][32 * j:32 * j + 1, :]
        row, col = 32 * (nt // 4), (nt % 4) * NQT
        gdst = gray_flat[row:row + 1, col:col + NQT]
        if nt % 2 == 0:
            nc.scalar.activation(gdst, psrc, AF.Copy)
        else:
            nc.vector.tensor_copy(gdst, psrc)
        eng = nc.sync if nt % 2 == 0 else nc.gpsimd
        eng.dma_start(scr1_2d[:, nt * NQT:(nt + 1) * NQT], gdst)
    sh = scr1.rearrange("(h w) -> h w", w=64)
    nc.gpsimd.dma_start(gray_img[:, :], sh[:, :])
    nc.gpsimd.dma_start(g_p1[0:63, :], sh[1:64, :])
    nc.gpsimd.dma_start(g_m1[1:64, :], sh[0:63, :])
    gvar = {1: g_p1, -1: g_m1}

    # ---- Laplacian + |.| on DVE (image layout) ----
    nc.vector.tensor_scalar(
        out=lap_t[:, :], in0=gray_img[:, :], scalar1=4.0, scalar2=None,
        op0=ALU.mult)
    for dy in (1, -1):  # out[h] += -g[h+dy]
        nc.vector.scalar_tensor_tensor(
            out=lap_t[:, :], in0=gvar[dy][:, :], scalar=-1.0, in1=lap_t[:, :],
            op0=ALU.mult, op1=ALU.add)
    for dx in (1, -1):
        c0, c1 = max(0, -dx), WW - max(0, dx)
        dst = lap_t[:, c0:c1]
        nc.vector.scalar_tensor_tensor(
            out=dst, in0=gray_img[:, c0 + dx:c1 + dx], scalar=-1.0, in1=dst,
            op0=ALU.mult, op1=ALU.add)
    nc.vector.scalar_tensor_tensor(
        out=abs_bf[:, :], in0=lap_t[:, :], scalar=-1.0, in1=lap_t[:, :],
        op0=ALU.mult, op1=ALU.max)

    # ---- af3: 3 row-shifted flat copies of |lap| (partition-gather DMAs);
    # the dx taps become column offsets at the matmul read, with the
    # baseline's accepted row-wrap artifact at image-row boundaries.
    for dyi, dy in enumerate((-1, 0, 1)):
        h0, h1 = max(0, -dy), 64 - max(0, dy)
        for hi, (a, b) in enumerate(((h0, 33), (33, h1))):
            eng = nc.sync if hi == 0 else nc.gpsimd
            eng.dma_start(
                af3[dyi:dyi + 1, 1 + a * 64:1 + b * 64],
                abs_bf[a + dy:b + dy, :])

    # ---- z0 = wG x(qtile0) + zb: query-side projection for qtile 0 ----
    pz = po_pool.tile([P, 2, NQT], f32, tag="po", name="pz")
    for mm in range(CC):
        nc.tensor.matmul(pz[:, mm, :], wg8_sb[:, :, mm * P:(mm + 1) * P],
                         xsl(x8_sb, 0, NQT), start=True, stop=True,
                         perf_mode=DR)
    z_t0 = qt_pool.tile([P, CC, NQT], f8)
    for mm in range(CC):
        nc.scalar.activation(z_t0[:, mm, :], pz[:, mm, :], AF.Identity,
                             bias=zb_sb[:, mm:mm + 1])

    # ---- vproj = (Wout @ Wv) x, transposed [nk, c], chunk PAIRS (fp8),
    # interleaved around the conv matmuls so the PE never waits on the
    # density chain's DMA latency.
    def vproj_pair(j2):
        pv = po_pool.tile([P, 2, C], f32, tag="po", name=f"pv{j2}")
        for u in range(2):
            nc.tensor.matmul(
                pv[:, u, :], xsl(x8_sb, (2 * j2 + u) * P, P),
                wvo8_sb[:, :, :], start=True, stop=True, perf_mode=DR)
        dstv = vproj_sb[:, 2 * j2:2 * j2 + 2, 0:C]
        if j2 % 2 == 1:
            nc.vector.tensor_copy(dstv, pv[:, :, :])
        else:
            nc.scalar.activation(dstv, pv[:, :, :], AF.Copy)

    for j2 in range(8):
        vproj_pair(j2)
    pe_warm(45)

    # ---- conv1: 3 accumulating dx-offset matmuls per query tile, col-tiled
    pcs = []
    for g in range(2):
        pc = po_pool.tile([P, NQT], f32, tag="po", name=f"pc{g}")
        for j in range(4):
            nt = 4 * g + j
            for dxi in range(3):
                nc.tensor.matmul(
                    pc[32 * j:32 * j + 8, :], w3x_sb[dxi][:, :],
                    af3[:, nt * NQT + dxi:nt * NQT + dxi + NQT],
                    start=(dxi == 0), stop=(dxi == 2),
                    tile_position=(0, 32 * j))
        pcs.append(pc)
    for nt in range(NQ_TILES):
        g, j = nt // 4, nt % 4
        src = pcs[g][32 * j:32 * j + 8, :]
        dst = h24[0:8, 1 + nt * NQT:1 + (nt + 1) * NQT]
        if nt % 2 == 0:
            nc.scalar.activation(dst, src, AF.Relu,
                                 bias=w1b128_sb[32 * j:32 * j + 8, 0:1])
        else:
            nc.vector.tensor_scalar(
                out=dst, in0=src, scalar1=w1b128_sb[32 * j:32 * j + 8, 0:1],
                scalar2=0.0, op0=ALU.add, op1=ALU.max)

    # ---- h24 rows 8:24: +-64-shifted copies of the relu block (rows 0:8,
    # written in place by the evicts); w24x row order matches (0, -1, +1)
    for bi, dy in enumerate((-1, 1)):
        off = 64 * dy
        i0, i1 = max(0, -off), N - max(0, off)
        im = (i0 + i1) // 2
        for hi, (a, b) in enumerate(((i0, im), (im, i1))):
            eng = nc.sync if hi == 0 else nc.gpsimd
            eng.dma_start(
                h24[8 + bi * 8:16 + bi * 8, 1 + a:1 + b],
                h24[0:8, 1 + a + off:1 + b + off])

    for j2 in range(8, NPAIR):
        vproj_pair(j2)
    pe_warm(45)

    # ---- conv2: 3 accumulating dx-offset matmuls per query tile, col-tiled
    pds = []
    for g in range(2):
        pd = po_pool.tile([P, NQT], f32, tag="po", name=f"pd{g}")
        for j in range(4):
            nt = 4 * g + j
            for dxi in range(3):
                nc.tensor.matmul(
                    pd[32 * j:32 * j + 1, :], w24x_sb[dxi][:, :],
                    h24[:, nt * NQT + dxi:nt * NQT + dxi + NQT],
                    start=(dxi == 0), stop=(dxi == 2),
                    tile_position=(0, 32 * j))
        pds.append(pd)
    for nt in range(NQ_TILES):
        g, j = nt // 4, nt % 4
        src = pds[g][32 * j:32 * j + 1, :]
        row, col = 32 * (nt // 4), (nt % 4) * NQT
        dst = dsum_flat[row:row + 1, col:col + NQT]
        if nt % 2 == 0:
            nc.scalar.activation(dst, src, AF.Copy)
        else:
            nc.vector.tensor_copy(dst, src)
        eng = nc.sync if nt % 2 == 0 else nc.gpsimd
        eng.dma_start(scr2_2d[:, nt * NQT:(nt + 1) * NQT], dst)
    # dsum image readback; sigmoid + skv = 1/(3-2*sig) in image layout;
    # skv -> DRAM flat via a CONTIGUOUS row-major write, then broadcast
    nc.gpsimd.dma_start(dsum_img[:, :], scr2.rearrange("(h w) -> h w", w=64))
    nc.scalar.activation(sig_img[:, :], dsum_img[:, :], AF.Sigmoid,
                         bias=w2b128_sb[0:64, 0:1])
    nc.vector.tensor_scalar(out=sig_img[:, :], in0=sig_img[:, :], scalar1=-2.0,
                            scalar2=3.0, op0=ALU.mult, op1=ALU.add)
    with nc.allow_low_precision(reason="skv in bf16: 0.4% exp-arg error"):
        nc.vector.reciprocal(skv_imgbf[:, :], sig_img[:, :])
    nc.sync.dma_start(scr2b.rearrange("(h w) -> h w", w=64), skv_imgbf[:, :])
    scr2b_2d = scr2b.rearrange("(a b) -> a b", a=1)
    for t in range(4):
        eng = nc.sync if t % 2 == 0 else nc.gpsimd
        eng.dma_start(
            skv_bc[:, 0, t * N // 4:(t + 1) * N // 4],
            scr2b_2d[0:1, t * N // 4:(t + 1) * N // 4]
            .broadcast_to([P, N // 4]))

    if has_r:
        for it in range(NQ_TILES):
            nc.sync.dma_start(
                r_sb[:, it, :],
                d["r"].rearrange("(a b) -> a b", a=1)
                [0:1, it * NQT:(it + 1) * NQT].broadcast_to([P, NQT]))

    pe_warm(10)

    # ---- f32 x for the residual add: emitted LAST so its 4MB transfer
    # never delays the density-chain DMAs (needed only from qtile 0's fin);
    # quarter descriptors so fin(qtile 0) sees its data early
    x_src = x_d.rearrange("(c p) n -> p c n", p=P)
    for t in range(4):
        nc.sync.dma_start(
            x_sb[:, :, t * N // 4:(t + 1) * N // 4],
            x_src[:, :, t * N // 4:(t + 1) * N // 4])

    pe_warm(120)

    # ---- attention: ONE flat 128-step pipeline over (qtile, key pair) ----
    z_tiles = [None] * NQ_TILES
    z_tiles[0] = z_t0
    for itn in range(1, NQ_TILES):
        z_nxt = qt_pool.tile([P, CC, NQT], f8)
        for mm in range(CC):
            pq = ps_pool.tile([P, NQT], f32, tag="ps")
            nc.tensor.matmul(
                pq[:, :], wg8_sb[:, :, mm * P:(mm + 1) * P],
                xsl(x8_sb, itn * NQT, NQT),
                start=True, stop=True, perf_mode=DR)
            nc.scalar.activation(z_nxt[:, mm, :], pq[:, :], AF.Identity,
                                 bias=zb_sb[:, mm:mm + 1])
        z_tiles[itn] = z_nxt
    pos_of = {}
    osb_of = {}

    def exp_tile(it):
        return exp_a if it % 2 == 0 else exp_b

    def attnv_pair(it, jj):
        pos, e = pos_of[it], exp_tile(it)
        for s in range(NSUB):
            nc.tensor.matmul(
                pos[s // 2][:, s % 2, 0:C + 1],
                e[:, 2 * jj:2 * jj + 2, s * P:(s + 1) * P],
                vproj_sb[:, 2 * jj:2 * jj + 2, :],
                start=(jj == 0), stop=(jj == NPAIR - 1),
                perf_mode=DR)

    def renorm(it):
        pos = pos_of.pop(it)
        rcp = rcp_pool.tile([P, NSUB // 2, 2, 1], f32)
        osb = osb_pool.tile([P, NSUB, C], bf16)
        for s2 in range(NSUB // 2):
            nc.vector.reciprocal(rcp[:, s2, :, :], pos[s2][:, :, C:C + 1])
            nc.vector.tensor_mul(
                osb[:, 2 * s2:2 * s2 + 2, :], pos[s2][:, :, 0:C],
                rcp[:, s2, :, :].broadcast_to([P, 2, C]))
        osb_of[it] = osb

    def finalize(it):
        osb, nq0 = osb_of.pop(it), it * NQT
        pt2 = ps_pool.tile([P, CC, NQT], bf16, tag="ps", name="pt")
        for ci in range(CC):
            for s in range(NSUB):
                nc.tensor.transpose(
                    pt2[:, ci, s * P:(s + 1) * P], osb[:, s, ci * P:(ci + 1) * P],
                    ident_bf[:, :])
        fin = fin_pool.tile([P, CC, NQT], f32)
        for ci in range(CC):
            nc.vector.scalar_tensor_tensor(
                out=fin[:, ci, :], in0=pt2[:, ci, :],
                scalar=bfin_sb[:, ci:ci + 1],
                in1=xsl(x_sb, nq0, NQT)[:, ci, :],
                op0=ALU.add, op1=ALU.add)
            nc.sync.dma_start(out_d[ci * P:(ci + 1) * P, nq0:nq0 + NQT], fin[:, ci, :])

    STEPS = [(it, jj) for it in range(NQ_TILES) for jj in range(NPAIR)]

    def av_step(idx):
        pit, pjj = STEPS[idx]
        attnv_pair(pit, pjj)
        if pjj == NPAIR - 1:
            renorm(pit)

    for idx, (it, jj) in enumerate(STEPS):
        if jj == 0:
            pos_of[it] = [po_pool.tile([P, 2, NQT], f32, tag="po",
                                       name=f"po{it}_{s2}")
                          for s2 in range(NSUB // 2)]
        if it == 0 and jj % 2 == 0:
            slx = slice((jj // 2) * NQT, (jj // 2 + 1) * NQT)
            nc.vector.tensor_mul(
                x8s_sb[:, :, slx], x8_sb[:, :, slx],
                skv_bc[:, :, slx].broadcast_to([P, CC, NQT]))
        e = exp_tile(it)
        ps2 = ps_pool.tile([P, 2, NQT], f32, tag="ps")
        for u in range(2):
            j = 2 * jj + u
            nc.tensor.matmul(
                ps2[:, u, :], xsl(x8s_sb, j * P, P), z_tiles[it][:, :, :],
                start=True, stop=True, perf_mode=DR)
        if has_r:
            for u in range(2):
                nc.vector.scalar_tensor_tensor(
                    out=ps2[:, u, :], in0=ps2[:, u, :], scalar=1.0,
                    in1=r_sb[:, it, :], op0=ALU.mult, op1=ALU.add)
        if jj in DVE_PAIRS:
            nc.vector.tensor_scalar(
                out=e[:, 2 * jj:2 * jj + 2, :].bitcast(u8), in0=ps2[:, :, :],
                scalar1=SCH_A, scalar2=SCH_B, op0=ALU.mult, op1=ALU.add)
        else:
            nc.scalar.activation(
                e[:, 2 * jj:2 * jj + 2, :], ps2[:, :, :], AF.Exp,
                bias=negb_sb[:, 0:1], scale=1.0 / 16.0)
        if idx >= AV_LAG:
            av_step(idx - AV_LAG)
        if jj == 7 and it >= 1:
            finalize(it - 1)
    for idx in range(len(STEPS) - AV_LAG, len(STEPS) - 1):
        av_step(idx)
    attnv_pair(NQ_TILES - 1, NPAIR - 1)
    it7, nq0 = NQ_TILES - 1, (NQ_TILES - 1) * NQT
    pos = pos_of.pop(it7)
    rcp = rcp_pool.tile([P, NSUB // 2, 2, 1], f32)
    osb = osb_pool.tile([P, NSUB, C], bf16)
    pt7 = ps_pool.tile([P, CC, NQT], bf16, tag="ps", name="pt7")
    fin7 = fin_pool.tile([P, CC, NQT], f32)
    for s2 in range(NSUB // 2):
        nc.vector.reciprocal(rcp[:, s2, :, :], pos[s2][:, :, C:C + 1])
        nc.vector.tensor_mul(
            osb[:, 2 * s2:2 * s2 + 2, :], pos[s2][:, :, 0:C],
            rcp[:, s2, :, :].broadcast_to([P, 2, C]))
        for ci in range(CC):
            for s in (2 * s2, 2 * s2 + 1):
                nc.tensor.transpose(
                    pt7[:, ci, s * P:(s + 1) * P],
                    osb[:, s, ci * P:(ci + 1) * P], ident_bf[:, :])
    for ci in range(CC):
        for hf in range(2):
            c0 = hf * (NQT // 2)
            nc.vector.scalar_tensor_tensor(
                out=fin7[:, ci, c0:c0 + NQT // 2],
                in0=pt7[:, ci, c0:c0 + NQT // 2],
                scalar=bfin_sb[:, ci:ci + 1],
                in1=xsl(x_sb, nq0 + c0, NQT // 2)[:, ci, :],
                op0=ALU.add, op1=ALU.add)
            nc.sync.dma_start(
                out_d[ci * P:(ci + 1) * P, nq0 + c0:nq0 + c0 + NQT // 2],
                fin7[:, ci, c0:c0 + NQT // 2])


def build_nc(has_r):
    nc = bacc.Bacc("TRN2", target_bir_lowering=False, debug=False)
    d = {}
    def inp(name, shape, dt=f32):
        d[name] = nc.dram_tensor(name, shape, dt, kind="ExternalInput").ap()
    inp("x", (C, N))
    inp("x8", (C, N), f8)
    inp("wf8", (C, 2 * C), f8)
    inp("qb4", (P, 4))
    inp("w9", (9, 8), bf16)
    inp("w72", (72, 1), bf16)
    inp("w1b128", (P, 1))
    inp("w2b128", (P, 1))
    if has_r:
        inp("r", (N,))
    d["out"] = nc.dram_tensor("out", (C, N), f32, kind="ExternalOutput").ap()
    d["scr1"] = nc.dram_tensor("scr1", (N,), bf16, kind="Internal").ap()
    d["scr2"] = nc.dram_tensor("scr2", (N,), bf16, kind="Internal").ap()
    d["scr2b"] = nc.dram_tensor("scr2b", (N,), bf16, kind="Internal").ap()

    with tile.TileContext(nc) as tc, ExitStack() as ctx:
        build_kernel_body(tc, ctx, d, has_r)
    nc.compile()
    return nc


def host_inputs(x, qkv_w, qkv_b, out_w, out_b, d1_w, d1_b, d2_w, d2_b):
    f = np.float32
    f8np = ml_dtypes.float8_e4m3
    bf = ml_dtypes.bfloat16
    x = np.asarray(x, f)
    wq = np.asarray(qkv_w, f)[:, :, 0, 0]          # [768, 256]
    qkv_b = np.asarray(qkv_b, f)
    wout = np.asarray(out_w, f)[:, :, 0, 0]        # [256, 256]
    out_b = np.asarray(out_b, f)
    wq_m, wk_m, wv_m = wq[0:C], wq[C:2 * C], wq[2 * C:3 * C]
    bq, bk, bv = qkv_b[0:C], qkv_b[C:2 * C], qkv_b[2 * C:3 * C]
    wG = wq_m.T @ wk_m                             # z = wG^T x + zb
    zb = wk_m.T @ bq                               # [256]
    wf8 = np.concatenate([wG, (wout @ wv_m).T], axis=1)
    qb4 = np.concatenate(
        [zb.reshape(2, P).T, (wout @ bv + out_b).reshape(2, P).T], axis=1)
    w1b128 = np.zeros((P, 1), f)
    for j in range(4):
        w1b128[32 * j:32 * j + 8, 0] = np.asarray(d1_b, f)
    shared = {
        "wf8": np.ascontiguousarray(wf8).astype(f8np),
        "qb4": np.ascontiguousarray(qb4, dtype=f),
        # dx-major conv weights: w9[dx*3+dy, oc], w72[dx*24+dy*8+ic]
        "w9": np.ascontiguousarray(
            np.transpose(np.asarray(d1_w, f)[:, 0], (2, 1, 0)).reshape(9, 8)
        ).astype(bf),
        "w72": np.ascontiguousarray(
            np.transpose(np.asarray(d2_w, f)[0], (2, 1, 0))[:, [1, 0, 2]]
            .reshape(72, 1)).astype(bf),
        "w1b128": w1b128,
        "w2b128": np.full((P, 1), np.asarray(d2_b, f).reshape(-1)[0], f),
    }
    xs = x.reshape(B, C, N)
    has_r = bool(np.abs(bk).max() > 0)
    maps = []
    for b in range(B):
        m = dict(x=np.ascontiguousarray(xs[b]),
                 x8=np.ascontiguousarray(xs[b]).astype(f8np), **shared)
        if has_r:
            m["r"] = np.ascontiguousarray(
                bk @ (wq_m @ xs[b]) + float(bk @ bq), dtype=f)
        maps.append(m)
    return maps, has_r


_NC_CACHE = {}


def _get_nc(has_r):
    key = ("nc", has_r)
    if key not in _NC_CACHE:
        _NC_CACHE[key] = build_nc(has_r)
    return _NC_CACHE[key]


def kernel(x, qkv_w, qkv_b, out_w, out_b, d1_w, d1_b, d2_w, d2_b):
    in_maps, has_r = host_inputs(
        x, qkv_w, qkv_b, out_w, out_b, d1_w, d1_b, d2_w, d2_b)
    nc = _get_nc(has_r)
    trace = bool(int(os.environ.get("KERNEL_TRACE", "0")))
    res = bass_utils.run_bass_kernel_spmd(
        nc, in_maps, core_ids=list(range(B)), trace=trace)
    _NC_CACHE["last_results"] = res
    out = np.stack([res.results[b]["out"] for b in range(B)])
    return np.ascontiguousarray(out.reshape(B, C, HH, WW).astype(np.float32))
